# revision 50
# baseline (speedup 1.0000x reference)
"""Trainium2 Bass kernel for nn_FCLModule_74131135529089 (moe_routing).

Module structure (B=262144 rows, input dim 1):
    circle/rect expert towers 1->32->64->256 (relu, zero biases)
    per-row select by shape_type, stage2 256->256 relu + residual,
    stage3 256->512 relu, 512->512, LayerNorm(512).

All bias vectors in this module are zero and every stage before the
LayerNorm is therefore positively homogeneous in x: for each row,
    h2(x) = |x| * H[k],   k = 2*shape_type + (x < 0),
where H[k] in R^512 is the full pre-LayerNorm output of the network
evaluated at x = +-1 for each expert.  The LayerNorm then collapses to
    out = C[k] * t + ln_b,  C[k] = (H[k]-mean(H[k]))*ln_g,
    t = |x| / sqrt(x^2 * var(H[k]) + eps).
The device kernel computes t/masks per row, forms a [rows,K] one-hot*t
matrix per 128-row chunk (fp16 hi/lo split rows, K padded to 32) and
multiplies it with the constant [K,512] matrix (C rows and ln_b) on the
tensor engine, streaming the 512 MB output at the HBM roofline.  If any
structural assumption is violated (nonzero biases / shape_type outside
{0,1}) we fall back to a dense numpy evaluation of the module.

Sharding: pure data parallel over the batch dim, 8 cores x 32768 rows.
"""

import numpy as np

B = 262144
TD = 512
N_CORES = 8
RPC = B // N_CORES          # rows per core = 32768
P = 128                     # SBUF partitions
CPB = RPC // P              # columns per partition = 256 (row r = p*CPB + j)
G = 8                       # 128-row chunks per output DMA (2 MB per DMA)
EPS = 1e-5
# Matmul operands are fp16 with an error-compensating split:
#   t*C = t_hi@C_hi + t_hi@C_lo + t_lo@C_hi   (+ ln_b hi/lo)
# folded into one K=14 matmul; fp16 streams at full PE rate (fp32 is 4x
# slower, float32r is ~11-bit) and the split recovers ~22 mantissa bits.
# K is padded to 32 so four 128-row chunks batch into one [128,128]
# PE transpose and run as four concurrent row-tiled (tile_position)
# K=32 matmuls against a 4x-replicated constant matrix.
KDIM = 32
TB = 4                      # chunks per transpose batch

_CACHE: dict = {}


def _towers_collapse(inputs):
    """Host-side constant folding (float64): returns the replicated fp16
    constant matrix [128,TD] and sig2 [4] f64, for k = 2*shape_type + (x<0)
    in order (c,+),(c,-),(r,+),(r,-)."""
    W = {k: np.asarray(v, dtype=np.float64) for k, v in inputs.items()}
    H = []
    for e in ("c", "r"):
        for sign in (1.0, -1.0):
            v = np.array([[sign]])
            for li in ("1", "2", "3"):
                v = np.maximum(v @ W[e + "w" + li] + W[e + "b" + li], 0.0)
            x2 = np.maximum(v @ W["s2w"] + W["s2b"], 0.0) + v
            h = np.maximum(x2 @ W["w3a"] + W["b3a"], 0.0)
            H.append((h @ W["w3b"] + W["b3b"])[0])
    # reorder to k = 2*s + neg: (c,+),(c,-),(r,+),(r,-) == H[0],H[1],H[2],H[3]
    H = np.stack(H)                                   # [4, TD]
    mu = H.mean(axis=1, keepdims=True)
    sig2 = H.var(axis=1)                              # [4]
    C = (H - mu) * W["ln_g"][None, :]                 # [4, TD]
    lnb = W["ln_b"]
    C_hi = C.astype(np.float16)
    C_lo = (C - C_hi.astype(np.float64)).astype(np.float16)
    b_hi = lnb.astype(np.float16)
    b_lo = (lnb - b_hi.astype(np.float64)).astype(np.float16)
    # rows: 0-3 C_hi (x t_hi), 4-7 C_lo (x t_hi), 8-11 C_hi (x t_lo),
    #       12 b_hi (x 1), 13 b_lo (x 1), 14-31 zero pad
    cmat = np.zeros((KDIM, TD), np.float16)
    cmat[0:4] = C_hi
    cmat[4:8] = C_lo
    cmat[8:12] = C_hi
    cmat[12] = b_hi
    cmat[13] = b_lo
    # replicated 4x along partitions for the row-tiled matmuls
    cmat_rep = np.tile(cmat, (TB, 1))                 # [128, TD] f16
    return np.ascontiguousarray(cmat_rep), sig2


def _assumptions_hold(inputs):
    for name in ("cb1", "cb2", "cb3", "rb1", "rb2", "rb3", "s2b", "b3a", "b3b"):
        if np.any(np.asarray(inputs[name]) != 0):
            return False
    st = np.asarray(inputs["shape_type"])
    if not np.isin(st, (0, 1)).all():
        return False
    x = np.asarray(inputs["x"])
    return bool(np.isfinite(x).all()) and x.shape == (B, 1) and st.shape == (B, 1)


def _fallback_numpy(inputs):
    f = {k: np.asarray(v, dtype=np.float32) for k, v in inputs.items()}

    def tower(h, w1, b1, w2, b2, w3, b3):
        h = np.maximum(h @ w1 + b1, 0)
        h = np.maximum(h @ w2 + b2, 0)
        return np.maximum(h @ w3 + b3, 0)

    x = f["x"]
    circle = tower(x, f["cw1"], f["cb1"], f["cw2"], f["cb2"], f["cw3"], f["cb3"])
    rect = tower(x, f["rw1"], f["rb1"], f["rw2"], f["rb2"], f["rw3"], f["rb3"])
    mask = np.asarray(inputs["shape_type"]) < 0.5
    x1 = np.where(mask, circle, rect)
    x2 = np.maximum(x1 @ f["s2w"] + f["s2b"], 0) + x1
    h = np.maximum(x2 @ f["w3a"] + f["b3a"], 0)
    h = h @ f["w3b"] + f["b3b"]
    mu = h.mean(axis=-1, keepdims=True)
    var = h.var(axis=-1, keepdims=True)
    return ((h - mu) / np.sqrt(var + EPS) * f["ln_g"] + f["ln_b"]).astype(np.float32)


def _build_nc(sig2, reps=1):
    import concourse.bacc as bacc
    import concourse.bass as bass
    import concourse.mybir as mybir
    import concourse.tile as tile

    f32 = mybir.dt.float32
    f16 = mybir.dt.float16
    a = float(sig2[0])
    b = float(sig2[1] - sig2[0])
    c = float(sig2[2] - sig2[0])
    d = float(sig2[3] - sig2[2] - sig2[1] + sig2[0])
    mul = mybir.AluOpType.mult
    add = mybir.AluOpType.add
    sub = mybir.AluOpType.subtract

    nc = bacc.Bacc("TRN2", target_bir_lowering=False, debug=False,
                   num_devices=N_CORES)
    x_d = nc.dram_tensor("x", [P, CPB], f32, kind="ExternalInput").ap()
    s_d = nc.dram_tensor("st", [P, CPB], f32, kind="ExternalInput").ap()
    c_d = nc.dram_tensor("cmat", [P, TD], f16, kind="ExternalInput").ap()
    i_d = nc.dram_tensor("ident", [P, P], f16, kind="ExternalInput").ap()
    y_d = nc.dram_tensor("y", [P, CPB, TD], f32, kind="ExternalOutput").ap()

    with tile.TileContext(nc) as tc:
        with (
            tc.tile_pool(name="const", bufs=1) as const,
            tc.tile_pool(name="pre", bufs=1) as pre,
            tc.tile_pool(name="lhs", bufs=6) as lhsp,
            tc.tile_pool(name="outs", bufs=3) as outp,
            tc.tile_pool(name="ps_t", bufs=2, space="PSUM") as ps_t,
            tc.tile_pool(name="ps_o", bufs=4, space="PSUM") as ps_o,
        ):
            xr = pre.tile([P, CPB], f32)
            sf = pre.tile([P, CPB], f32)
            neg = pre.tile([P, CPB], f32)
            sn = pre.tile([P, CPB], f32)
            u1 = pre.tile([P, CPB], f32)
            u2 = pre.tile([P, CPB], f32)
            sg = pre.tile([P, CPB], f32)
            x2 = pre.tile([P, CPB], f32)
            ve = pre.tile([P, CPB], f32)
            rc = pre.tile([P, CPB], f32)
            t2 = pre.tile([P, CPB], f32)
            tt = pre.tile([P, CPB], f32)
            th16 = pre.tile([P, CPB], f16)
            th32 = pre.tile([P, CPB], f32)
            tl = pre.tile([P, CPB], f32)
            m1 = pre.tile([P, CPB], f32)
            m2 = pre.tile([P, CPB], f32)
            w0 = pre.tile([P, CPB], f32)
            m0 = pre.tile([P, CPB], f32)
            m4 = pre.tile([P, CPB, KDIM], f16)

            def emit_group(g0, gsz):
                outt = outp.tile([P, G, TD], f32, tag="outt")
                for j4 in range(0, gsz, TB):
                    nb = min(TB, gsz - j4)
                    tp = ps_t.tile([P, P], f16, tag="tp")
                    nc.tensor.transpose(
                        tp[:KDIM * nb],
                        m4[:, g0 + j4:g0 + j4 + nb, :], idt[:])
                    lh = lhsp.tile([P, P], f16, tag="lh")
                    nc.vector.tensor_copy(lh[:KDIM * nb], tp[:KDIM * nb])
                    for j in range(nb):
                        po = ps_o.tile([P, TD], f32, tag="po")
                        nc.tensor.matmul(
                            po[:], lh[KDIM * j:KDIM * (j + 1), :],
                            cm[KDIM * j:KDIM * (j + 1), :],
                            start=True, stop=True,
                            tile_position=(KDIM * j, 0))
                        nc.scalar.copy(outt[:, j4 + j, :], po[:])
                nc.sync.dma_start(y_d[:, g0:g0 + gsz, :], outt[:, 0:gsz, :])

            nc.sync.dma_start(xr[:], x_d[:])
            nc.scalar.dma_start(sf[:], s_d[:])
            cm = const.tile([P, TD], f16)
            nc.scalar.dma_start(cm[:], c_d[:])
            idt = const.tile([P, P], f16)
            nc.sync.dma_start(idt[:], i_d[:])
            # pad planes (14-31) contribute via zero rows of cmat, but
            # must not hold NaN garbage; zeroed on GpSimd off the DVE
            # critical path (tiny first piece so early chunks aren't
            # gated by the big memset).
            nc.gpsimd.memset(m4[:, 0:8, 14:KDIM], 0.0)
            nc.gpsimd.memset(m4[:, 8:CPB, 14:KDIM], 0.0)

            # preprocessing in graded column slices so the chunk pipeline
            # can start as soon as the first small slice of m4 is ready.
            # Each slice's prep is emitted one slice AHEAD of the previous
            # slice's output groups: the sqrt is an ACT op and must sit in
            # ACT's strict FIFO before the (long) block of output copies,
            # or the next slice's chunks starve the DMA stream.
            bounds = [0, 8, 24, 64, 128, 192, CPB]
            nslices = len(bounds) - 1

            def emit_prep(h):
                cs = slice(bounds[h], bounds[h + 1])
                nc.vector.tensor_scalar(neg[:, cs], xr[:, cs], 0.0, None,
                                        mybir.AluOpType.is_lt)
                nc.vector.tensor_tensor(sn[:, cs], sf[:, cs], neg[:, cs], mul)
                # sig2 per row: a + b*neg + c*sf + d*sn
                nc.vector.tensor_scalar(u1[:, cs], sf[:, cs], c, a, mul, add)
                nc.vector.scalar_tensor_tensor(u2[:, cs], neg[:, cs], b,
                                               u1[:, cs], mul, add)
                nc.vector.scalar_tensor_tensor(sg[:, cs], sn[:, cs], d,
                                               u2[:, cs], mul, add)
                # t = sqrt(x^2 / (x^2*sig2 + eps))
                nc.vector.tensor_tensor(x2[:, cs], xr[:, cs], xr[:, cs], mul)
                nc.vector.tensor_tensor(ve[:, cs], x2[:, cs], sg[:, cs], mul)
                nc.vector.tensor_scalar(ve[:, cs], ve[:, cs], EPS, None, add)
                nc.vector.reciprocal(rc[:, cs], ve[:, cs])
                nc.vector.tensor_tensor(t2[:, cs], x2[:, cs], rc[:, cs], mul)
                nc.scalar.activation(tt[:, cs], t2[:, cs],
                                     mybir.ActivationFunctionType.Sqrt)
                # split t = t_hi + t_lo (t_hi exactly fp16-representable)
                nc.vector.tensor_copy(th16[:, cs], tt[:, cs])
                nc.vector.tensor_copy(th32[:, cs], th16[:, cs])
                nc.vector.tensor_tensor(tl[:, cs], tt[:, cs], th32[:, cs], sub)
                # masks: m0=(1-s)(1-n), m1=(1-s)n, m2=s(1-n), m3=s*n
                nc.vector.tensor_tensor(m1[:, cs], neg[:, cs], sn[:, cs], sub)
                nc.vector.tensor_tensor(m2[:, cs], sf[:, cs], sn[:, cs], sub)
                nc.vector.tensor_scalar(w0[:, cs], sf[:, cs], -1.0, 1.0,
                                        mul, add)
                nc.vector.tensor_tensor(m0[:, cs], w0[:, cs], m1[:, cs], sub)
                for j, mj in enumerate((m0, m1, m2, sn)):
                    nc.vector.tensor_tensor(m4[:, cs, j], tt[:, cs],
                                            mj[:, cs], mul)
                    nc.vector.tensor_tensor(m4[:, cs, 4 + j], tt[:, cs],
                                            mj[:, cs], mul)
                    nc.vector.tensor_tensor(m4[:, cs, 8 + j], tl[:, cs],
                                            mj[:, cs], mul)
                nc.vector.memset(m4[:, cs, 12], 1.0)
                nc.vector.memset(m4[:, cs, 13], 1.0)

            def emit_groups(h):
                # ramped at the very start so the first DMA goes out early
                if h == 0:
                    groups = [(0, 2), (2, 2), (4, 4)]
                else:
                    groups = [(g, G) for g in range(bounds[h], bounds[h + 1], G)]
                for i, (g0, gsz) in enumerate(groups):
                    emit_group(g0, gsz)
                    if i == 0 and h + 1 < nslices:
                        # next slice's prep right after this slice's first
                        # group: its sqrt precedes the bulk of ACT copies
                        # in FIFO order, but the first chunks' copies are
                        # already queued ahead of it.
                        emit_prep(h + 1)

            emit_prep(0)
            for h in range(nslices):
                emit_groups(h)

            # extra full passes for repeat-based HW timing (reps > 1)
            for _ in range(reps - 1):
                for g0 in range(0, CPB, G):
                    emit_group(g0, G)
    nc.compile()
    return nc


def _make_in_maps(inputs, cmat):
    x = np.ascontiguousarray(np.asarray(inputs["x"], dtype=np.float32)).reshape(B)
    st = np.asarray(inputs["shape_type"]).astype(np.float32).reshape(B)
    ident = np.eye(P, dtype=np.float16)
    in_maps = []
    for i in range(N_CORES):
        sl = slice(i * RPC, (i + 1) * RPC)
        in_maps.append({
            "x": x[sl].reshape(P, CPB).copy(),
            "st": st[sl].reshape(P, CPB).copy(),
            "cmat": cmat,
            "ident": ident,
        })
    return in_maps


def _get_nc(sig2):
    key = tuple(np.round(sig2, 12))
    if key not in _CACHE:
        _CACHE[key] = _build_nc(sig2)
    return _CACHE[key]


def _get_runner(nc):
    """Cached jit-compiled SPMD executor for `nc` (same mechanics as
    concourse.bass2jax.run_bass_via_pjrt, memoized so repeated kernel()
    calls skip jax re-tracing)."""
    if hasattr(nc, "_cached_runner"):
        return nc._cached_runner
    import jax
    from jax.experimental.shard_map import shard_map
    from jax.sharding import Mesh, PartitionSpec

    import concourse.mybir as mybir
    from concourse import bass2jax

    bass2jax.install_neuronx_cc_hook()

    part_name = (nc.partition_id_tensor.name
                 if nc.partition_id_tensor else None)
    in_names, out_names, out_avals = [], [], []
    for alloc in nc.m.functions[0].allocations:
        if not isinstance(alloc, mybir.MemoryLocationSet):
            continue
        name = alloc.memorylocations[0].name
        if alloc.kind == "ExternalInput":
            if name != part_name:
                in_names.append(name)
        elif alloc.kind == "ExternalOutput":
            out_names.append(name)
            out_avals.append(jax.core.ShapedArray(
                tuple(alloc.tensor_shape), mybir.dt.np(alloc.dtype)))
    n_params = len(in_names)
    all_names = in_names + out_names
    if part_name is not None:
        all_names = all_names + [part_name]
    donate = tuple(range(n_params, n_params + len(out_names)))

    def _body(*args):
        operands = list(args)
        if part_name is not None:
            operands.append(bass2jax.partition_id_tensor())
        return tuple(bass2jax._bass_exec_p.bind(
            *operands,
            out_avals=tuple(out_avals),
            in_names=tuple(all_names),
            out_names=tuple(out_names),
            lowering_input_output_aliases=(),
            sim_require_finite=True,
            sim_require_nnan=True,
            nc=nc,
        ))

    devices = jax.devices()[:N_CORES]
    mesh = Mesh(np.asarray(devices), ("core",))
    sharded = jax.jit(
        shard_map(_body, mesh=mesh,
                  in_specs=(PartitionSpec("core"),) * (n_params + len(out_names)),
                  out_specs=(PartitionSpec("core"),) * len(out_names),
                  check_rep=False),
        donate_argnums=donate, keep_unused=True)
    runner = (sharded, in_names, out_names, out_avals)
    nc._cached_runner = runner
    return runner


def _run_spmd(nc, in_maps):
    sharded, in_names, out_names, out_avals = _get_runner(nc)
    concat_in = [
        np.concatenate([np.asarray(m[name])[None] for m in in_maps], axis=0)
        .reshape(N_CORES * in_maps[0][name].shape[0],
                 *in_maps[0][name].shape[1:])
        for name in in_names
    ]
    concat_zeros = [
        np.zeros((N_CORES * a.shape[0], *a.shape[1:]), a.dtype)
        for a in out_avals
    ]
    out_arrs = sharded(*concat_in, *concat_zeros)
    return {
        name: np.asarray(out_arrs[i]).reshape(
            N_CORES, *out_avals[i].shape)
        for i, name in enumerate(out_names)
    }


def kernel(**inputs) -> np.ndarray:
    if not _assumptions_hold(inputs):
        return _fallback_numpy(inputs)

    cmat, sig2 = _towers_collapse(inputs)
    nc = _get_nc(sig2)
    in_maps = _make_in_maps(inputs, cmat)
    y = _run_spmd(nc, in_maps)["y"]            # [N_CORES, P, CPB, TD]
    return np.ascontiguousarray(y.reshape(B, TD))


# revision 54
# speedup vs baseline: 1.0188x; 1.0188x over previous
"""Trainium2 Bass kernel for nn_FCLModule_74131135529089 (moe_routing).

Module structure (B=262144 rows, input dim 1):
    circle/rect expert towers 1->32->64->256 (relu, zero biases)
    per-row select by shape_type, stage2 256->256 relu + residual,
    stage3 256->512 relu, 512->512, LayerNorm(512).

All bias vectors in this module are zero and every stage before the
LayerNorm is therefore positively homogeneous in x: for each row,
    h2(x) = |x| * H[k],   k = 2*shape_type + (x < 0),
where H[k] in R^512 is the full pre-LayerNorm output of the network
evaluated at x = +-1 for each expert.  The LayerNorm then collapses to
    out = C[k] * t + ln_b,  C[k] = (H[k]-mean(H[k]))*ln_g,
    t = |x| / sqrt(x^2 * var(H[k]) + eps).
The device kernel computes t/masks per row, forms a [rows,K] one-hot*t
matrix per 128-row chunk (fp16 hi/lo split rows, K padded to 32) and
multiplies it with the constant [K,512] matrix (C rows and ln_b) on the
tensor engine, streaming the 512 MB output at the HBM roofline.  If any
structural assumption is violated (nonzero biases / shape_type outside
{0,1}) we fall back to a dense numpy evaluation of the module.

Sharding: pure data parallel over the batch dim, 8 cores x 32768 rows.
"""

import numpy as np

B = 262144
TD = 512
N_CORES = 8
RPC = B // N_CORES          # rows per core = 32768
P = 128                     # SBUF partitions
CPB = RPC // P              # columns per partition = 256 (row r = p*CPB + j)
G = 16                      # 128-row chunks per output DMA (4 MB per DMA)
EPS = 1e-5
# Matmul operands are fp16 with an error-compensating split:
#   t*C = t_hi@C_hi + t_hi@C_lo + t_lo@C_hi   (+ ln_b hi/lo)
# folded into one K=14 matmul; fp16 streams at full PE rate (fp32 is 4x
# slower, float32r is ~11-bit) and the split recovers ~22 mantissa bits.
# K is padded to 32 so four 128-row chunks batch into one [128,128]
# PE transpose and run as four concurrent row-tiled (tile_position)
# K=32 matmuls against a 4x-replicated constant matrix.
KDIM = 32
TB = 4                      # chunks per transpose batch

_CACHE: dict = {}


def _towers_collapse(inputs):
    """Host-side constant folding (float64): returns the replicated fp16
    constant matrix [128,TD] and sig2 [4] f64, for k = 2*shape_type + (x<0)
    in order (c,+),(c,-),(r,+),(r,-)."""
    W = {k: np.asarray(v, dtype=np.float64) for k, v in inputs.items()}
    H = []
    for e in ("c", "r"):
        for sign in (1.0, -1.0):
            v = np.array([[sign]])
            for li in ("1", "2", "3"):
                v = np.maximum(v @ W[e + "w" + li] + W[e + "b" + li], 0.0)
            x2 = np.maximum(v @ W["s2w"] + W["s2b"], 0.0) + v
            h = np.maximum(x2 @ W["w3a"] + W["b3a"], 0.0)
            H.append((h @ W["w3b"] + W["b3b"])[0])
    # reorder to k = 2*s + neg: (c,+),(c,-),(r,+),(r,-) == H[0],H[1],H[2],H[3]
    H = np.stack(H)                                   # [4, TD]
    mu = H.mean(axis=1, keepdims=True)
    sig2 = H.var(axis=1)                              # [4]
    C = (H - mu) * W["ln_g"][None, :]                 # [4, TD]
    lnb = W["ln_b"]
    C_hi = C.astype(np.float16)
    C_lo = (C - C_hi.astype(np.float64)).astype(np.float16)
    b_hi = lnb.astype(np.float16)
    b_lo = (lnb - b_hi.astype(np.float64)).astype(np.float16)
    # rows: 0-3 C_hi (x t_hi), 4-7 C_lo (x t_hi), 8-11 C_hi (x t_lo),
    #       12 b_hi (x 1), 13 b_lo (x 1), 14-31 zero pad
    cmat = np.zeros((KDIM, TD), np.float16)
    cmat[0:4] = C_hi
    cmat[4:8] = C_lo
    cmat[8:12] = C_hi
    cmat[12] = b_hi
    cmat[13] = b_lo
    # replicated 4x along partitions for the row-tiled matmuls
    cmat_rep = np.tile(cmat, (TB, 1))                 # [128, TD] f16
    return np.ascontiguousarray(cmat_rep), sig2


def _assumptions_hold(inputs):
    for name in ("cb1", "cb2", "cb3", "rb1", "rb2", "rb3", "s2b", "b3a", "b3b"):
        if np.any(np.asarray(inputs[name]) != 0):
            return False
    st = np.asarray(inputs["shape_type"])
    if not np.isin(st, (0, 1)).all():
        return False
    x = np.asarray(inputs["x"])
    return bool(np.isfinite(x).all()) and x.shape == (B, 1) and st.shape == (B, 1)


def _fallback_numpy(inputs):
    f = {k: np.asarray(v, dtype=np.float32) for k, v in inputs.items()}

    def tower(h, w1, b1, w2, b2, w3, b3):
        h = np.maximum(h @ w1 + b1, 0)
        h = np.maximum(h @ w2 + b2, 0)
        return np.maximum(h @ w3 + b3, 0)

    x = f["x"]
    circle = tower(x, f["cw1"], f["cb1"], f["cw2"], f["cb2"], f["cw3"], f["cb3"])
    rect = tower(x, f["rw1"], f["rb1"], f["rw2"], f["rb2"], f["rw3"], f["rb3"])
    mask = np.asarray(inputs["shape_type"]) < 0.5
    x1 = np.where(mask, circle, rect)
    x2 = np.maximum(x1 @ f["s2w"] + f["s2b"], 0) + x1
    h = np.maximum(x2 @ f["w3a"] + f["b3a"], 0)
    h = h @ f["w3b"] + f["b3b"]
    mu = h.mean(axis=-1, keepdims=True)
    var = h.var(axis=-1, keepdims=True)
    return ((h - mu) / np.sqrt(var + EPS) * f["ln_g"] + f["ln_b"]).astype(np.float32)


def _build_nc(sig2, reps=1):
    import concourse.bacc as bacc
    import concourse.bass as bass
    import concourse.mybir as mybir
    import concourse.tile as tile

    f32 = mybir.dt.float32
    f16 = mybir.dt.float16
    a = float(sig2[0])
    b = float(sig2[1] - sig2[0])
    c = float(sig2[2] - sig2[0])
    d = float(sig2[3] - sig2[2] - sig2[1] + sig2[0])
    mul = mybir.AluOpType.mult
    add = mybir.AluOpType.add
    sub = mybir.AluOpType.subtract

    nc = bacc.Bacc("TRN2", target_bir_lowering=False, debug=False,
                   num_devices=N_CORES)
    x_d = nc.dram_tensor("x", [P, CPB], f32, kind="ExternalInput").ap()
    s_d = nc.dram_tensor("st", [P, CPB], f32, kind="ExternalInput").ap()
    c_d = nc.dram_tensor("cmat", [P, TD], f16, kind="ExternalInput").ap()
    i_d = nc.dram_tensor("ident", [P, P], f16, kind="ExternalInput").ap()
    y_d = nc.dram_tensor("y", [P, CPB, TD], f32, kind="ExternalOutput").ap()

    with tile.TileContext(nc) as tc:
        with (
            tc.tile_pool(name="const", bufs=1) as const,
            tc.tile_pool(name="pre", bufs=1) as pre,
            tc.tile_pool(name="lhs", bufs=6) as lhsp,
            tc.tile_pool(name="outs", bufs=3) as outp,
            tc.tile_pool(name="ps_t", bufs=2, space="PSUM") as ps_t,
            tc.tile_pool(name="ps_o", bufs=3, space="PSUM") as ps_o,
        ):
            xr = pre.tile([P, CPB], f32)
            sf = pre.tile([P, CPB], f32)
            neg = pre.tile([P, CPB], f32)
            sn = pre.tile([P, CPB], f32)
            u1 = pre.tile([P, CPB], f32)
            u2 = pre.tile([P, CPB], f32)
            sg = pre.tile([P, CPB], f32)
            x2 = pre.tile([P, CPB], f32)
            ve = pre.tile([P, CPB], f32)
            rc = pre.tile([P, CPB], f32)
            t2 = pre.tile([P, CPB], f32)
            tt = pre.tile([P, CPB], f32)
            th16 = pre.tile([P, CPB], f16)
            th32 = pre.tile([P, CPB], f32)
            tl = pre.tile([P, CPB], f32)
            m1 = pre.tile([P, CPB], f32)
            m2 = pre.tile([P, CPB], f32)
            w0 = pre.tile([P, CPB], f32)
            m0 = pre.tile([P, CPB], f32)
            m4 = pre.tile([P, CPB, KDIM], f16)

            def emit_group(g0, gsz):
                outt = outp.tile([P, G, TD], f32, tag="outt")
                for j4 in range(0, gsz, TB):
                    nb = min(TB, gsz - j4)
                    tp = ps_t.tile([P, P], f16, tag="tp")
                    nc.tensor.transpose(
                        tp[:KDIM * nb],
                        m4[:, g0 + j4:g0 + j4 + nb, :], idt[:])
                    lh = lhsp.tile([P, P], f16, tag="lh")
                    nc.vector.tensor_copy(lh[:KDIM * nb], tp[:KDIM * nb])
                    for j2 in range(0, nb, 2):
                        # two chunks' matmuls land in one 2-bank PSUM tile
                        # so a single ACT copy drains both
                        pp = ps_o.tile([P, 2, TD], f32, tag="po")
                        for j in (j2, j2 + 1):
                            nc.tensor.matmul(
                                pp[:, j - j2, :],
                                lh[KDIM * j:KDIM * (j + 1), :],
                                cm[KDIM * j:KDIM * (j + 1), :],
                                start=True, stop=True,
                                tile_position=(KDIM * j, 0))
                        nc.scalar.copy(outt[:, j4 + j2:j4 + j2 + 2, :], pp[:])
                nc.sync.dma_start(y_d[:, g0:g0 + gsz, :], outt[:, 0:gsz, :])

            nc.sync.dma_start(xr[:], x_d[:])
            nc.scalar.dma_start(sf[:], s_d[:])
            cm = const.tile([P, TD], f16)
            nc.scalar.dma_start(cm[:], c_d[:])
            idt = const.tile([P, P], f16)
            nc.sync.dma_start(idt[:], i_d[:])
            # pad planes (14-31) contribute via zero rows of cmat, but
            # must not hold NaN garbage; zeroed on GpSimd off the DVE
            # critical path (tiny first piece so early chunks aren't
            # gated by the big memset).
            nc.gpsimd.memset(m4[:, 0:8, 14:KDIM], 0.0)
            nc.gpsimd.memset(m4[:, 8:CPB, 14:KDIM], 0.0)

            # preprocessing in graded column slices so the chunk pipeline
            # can start as soon as the first small slice of m4 is ready.
            # Each slice's prep is emitted one slice AHEAD of the previous
            # slice's output groups: the sqrt is an ACT op and must sit in
            # ACT's strict FIFO before the (long) block of output copies,
            # or the next slice's chunks starve the DMA stream.
            bounds = [0, 8, 24, 64, 128, 192, CPB]
            nslices = len(bounds) - 1

            def emit_prep(h):
                cs = slice(bounds[h], bounds[h + 1])
                nc.vector.tensor_scalar(neg[:, cs], xr[:, cs], 0.0, None,
                                        mybir.AluOpType.is_lt)
                nc.vector.tensor_tensor(sn[:, cs], sf[:, cs], neg[:, cs], mul)
                # sig2 per row: a + b*neg + c*sf + d*sn
                nc.vector.tensor_scalar(u1[:, cs], sf[:, cs], c, a, mul, add)
                nc.vector.scalar_tensor_tensor(u2[:, cs], neg[:, cs], b,
                                               u1[:, cs], mul, add)
                nc.vector.scalar_tensor_tensor(sg[:, cs], sn[:, cs], d,
                                               u2[:, cs], mul, add)
                # t = sqrt(x^2 / (x^2*sig2 + eps))
                nc.vector.tensor_tensor(x2[:, cs], xr[:, cs], xr[:, cs], mul)
                nc.vector.tensor_tensor(ve[:, cs], x2[:, cs], sg[:, cs], mul)
                nc.vector.tensor_scalar(ve[:, cs], ve[:, cs], EPS, None, add)
                nc.vector.reciprocal(rc[:, cs], ve[:, cs])
                nc.vector.tensor_tensor(t2[:, cs], x2[:, cs], rc[:, cs], mul)
                nc.scalar.activation(tt[:, cs], t2[:, cs],
                                     mybir.ActivationFunctionType.Sqrt)
                # split t = t_hi + t_lo (t_hi exactly fp16-representable)
                nc.vector.tensor_copy(th16[:, cs], tt[:, cs])
                nc.vector.tensor_copy(th32[:, cs], th16[:, cs])
                nc.vector.tensor_tensor(tl[:, cs], tt[:, cs], th32[:, cs], sub)
                # masks: m0=(1-s)(1-n), m1=(1-s)n, m2=s(1-n), m3=s*n
                nc.vector.tensor_tensor(m1[:, cs], neg[:, cs], sn[:, cs], sub)
                nc.vector.tensor_tensor(m2[:, cs], sf[:, cs], sn[:, cs], sub)
                nc.vector.tensor_scalar(w0[:, cs], sf[:, cs], -1.0, 1.0,
                                        mul, add)
                nc.vector.tensor_tensor(m0[:, cs], w0[:, cs], m1[:, cs], sub)
                for j, mj in enumerate((m0, m1, m2, sn)):
                    nc.vector.tensor_tensor(m4[:, cs, j], tt[:, cs],
                                            mj[:, cs], mul)
                    nc.vector.tensor_tensor(m4[:, cs, 4 + j], tt[:, cs],
                                            mj[:, cs], mul)
                    nc.vector.tensor_tensor(m4[:, cs, 8 + j], tl[:, cs],
                                            mj[:, cs], mul)
                nc.vector.memset(m4[:, cs, 12], 1.0)
                nc.vector.memset(m4[:, cs, 13], 1.0)

            def emit_groups(h):
                # ramped at the very start so the first DMA goes out early
                if h == 0:
                    groups = [(0, 2), (2, 2), (4, 4)]
                else:
                    groups = []
                    g = bounds[h]
                    while g < bounds[h + 1]:
                        gsz = min(G, bounds[h + 1] - g)
                        groups.append((g, gsz))
                        g += gsz
                for i, (g0, gsz) in enumerate(groups):
                    emit_group(g0, gsz)
                    if i == 0 and h + 1 < nslices:
                        # next slice's prep right after this slice's first
                        # group: its sqrt precedes the bulk of ACT copies
                        # in FIFO order, but the first chunks' copies are
                        # already queued ahead of it.
                        emit_prep(h + 1)

            emit_prep(0)
            for h in range(nslices):
                emit_groups(h)

            # extra full passes for repeat-based HW timing (reps > 1)
            for _ in range(reps - 1):
                for g0 in range(0, CPB, G):
                    emit_group(g0, G)
    nc.compile()
    return nc


def _make_in_maps(inputs, cmat):
    x = np.ascontiguousarray(np.asarray(inputs["x"], dtype=np.float32)).reshape(B)
    st = np.asarray(inputs["shape_type"]).astype(np.float32).reshape(B)
    ident = np.eye(P, dtype=np.float16)
    in_maps = []
    for i in range(N_CORES):
        sl = slice(i * RPC, (i + 1) * RPC)
        in_maps.append({
            "x": x[sl].reshape(P, CPB).copy(),
            "st": st[sl].reshape(P, CPB).copy(),
            "cmat": cmat,
            "ident": ident,
        })
    return in_maps


def _get_nc(sig2):
    key = tuple(np.round(sig2, 12))
    if key not in _CACHE:
        _CACHE[key] = _build_nc(sig2)
    return _CACHE[key]


def _get_runner(nc):
    """Cached jit-compiled SPMD executor for `nc` (same mechanics as
    concourse.bass2jax.run_bass_via_pjrt, memoized so repeated kernel()
    calls skip jax re-tracing)."""
    if hasattr(nc, "_cached_runner"):
        return nc._cached_runner
    import jax
    from jax.experimental.shard_map import shard_map
    from jax.sharding import Mesh, PartitionSpec

    import concourse.mybir as mybir
    from concourse import bass2jax

    bass2jax.install_neuronx_cc_hook()

    part_name = (nc.partition_id_tensor.name
                 if nc.partition_id_tensor else None)
    in_names, out_names, out_avals = [], [], []
    for alloc in nc.m.functions[0].allocations:
        if not isinstance(alloc, mybir.MemoryLocationSet):
            continue
        name = alloc.memorylocations[0].name
        if alloc.kind == "ExternalInput":
            if name != part_name:
                in_names.append(name)
        elif alloc.kind == "ExternalOutput":
            out_names.append(name)
            out_avals.append(jax.core.ShapedArray(
                tuple(alloc.tensor_shape), mybir.dt.np(alloc.dtype)))
    n_params = len(in_names)
    all_names = in_names + out_names
    if part_name is not None:
        all_names = all_names + [part_name]
    donate = tuple(range(n_params, n_params + len(out_names)))

    def _body(*args):
        operands = list(args)
        if part_name is not None:
            operands.append(bass2jax.partition_id_tensor())
        return tuple(bass2jax._bass_exec_p.bind(
            *operands,
            out_avals=tuple(out_avals),
            in_names=tuple(all_names),
            out_names=tuple(out_names),
            lowering_input_output_aliases=(),
            sim_require_finite=True,
            sim_require_nnan=True,
            nc=nc,
        ))

    devices = jax.devices()[:N_CORES]
    mesh = Mesh(np.asarray(devices), ("core",))
    sharded = jax.jit(
        shard_map(_body, mesh=mesh,
                  in_specs=(PartitionSpec("core"),) * (n_params + len(out_names)),
                  out_specs=(PartitionSpec("core"),) * len(out_names),
                  check_rep=False),
        donate_argnums=donate, keep_unused=True)
    runner = (sharded, in_names, out_names, out_avals)
    nc._cached_runner = runner
    return runner


def _run_spmd(nc, in_maps):
    sharded, in_names, out_names, out_avals = _get_runner(nc)
    concat_in = [
        np.concatenate([np.asarray(m[name])[None] for m in in_maps], axis=0)
        .reshape(N_CORES * in_maps[0][name].shape[0],
                 *in_maps[0][name].shape[1:])
        for name in in_names
    ]
    concat_zeros = [
        np.zeros((N_CORES * a.shape[0], *a.shape[1:]), a.dtype)
        for a in out_avals
    ]
    out_arrs = sharded(*concat_in, *concat_zeros)
    return {
        name: np.asarray(out_arrs[i]).reshape(
            N_CORES, *out_avals[i].shape)
        for i, name in enumerate(out_names)
    }


def kernel(**inputs) -> np.ndarray:
    if not _assumptions_hold(inputs):
        return _fallback_numpy(inputs)

    cmat, sig2 = _towers_collapse(inputs)
    nc = _get_nc(sig2)
    in_maps = _make_in_maps(inputs, cmat)
    y = _run_spmd(nc, in_maps)["y"]            # [N_CORES, P, CPB, TD]
    return np.ascontiguousarray(y.reshape(B, TD))
